# revision 29
# baseline (speedup 1.0000x reference)
"""Trainium2 Bass kernel for nn_BKT_RNN: tanh-RNN (H=4) + BKT latent recurrence.

Math notes (validated to 1-2 ulp against the reference):
  * The reference's conditioning step telescopes: m_t = k1*c + k0*(1-c) == latent
    exactly, so the latent recurrence is LINEAR given the RNN outputs:
        latent' = latent*(1 - f - l) + l
        correct = latent*(1 - s - g) + g
  * (tanh(z)+1)/2 == sigmoid(2z), so the RNN state is kept in p-space:
        p' = sigmoid(2*(sum_j 2*W_hh[i,j]*p_j + W_ih[i]*x + (b_ih+b_hh-sum_j W_hh[i,j])))
  * BCE select: y in {0,1} -> y?c:1-c == 2*c*y - c - y + 1 (exact in fp32).

Sharding: data-parallel over students. Each of the 8 cores gets 512 students
(128 partitions x 4 groups); the full sequential T-loop runs per core; loss
partials are summed on the host.

Layout: IN is plane-major [p, (j, g, tau)] with j: 0-3 = state planes
(p-space), 4 = x, 5 = const 1. The sigmoid of step t writes the state planes
at tau=t+1; phase B reads l,f,g,s back from those planes. Group-major planes
keep every DMA and phase-B access contiguous along t.

The RNN loop runs as two independent chains (groups 0-1 and 2-3) so the
second chain's vector/scalar work hides the first chain's cross-engine
latency.
"""

import sys
from contextlib import ExitStack

import numpy as np

sys.path.insert(0, "/opt/trn_rl_repo")

T = 1024
B = 4096
H = 4
NCORES = 8
BS = B // NCORES  # students per core
P = 128
G = BS // P  # student groups per core (free dim)
C = 64  # phase-B chunk length (timesteps)
SEC = 256  # output DMA section length
NCHAIN = 2  # independent RNN chains (groups per chain = G // NCHAIN)

_PROGRAM = None


def _build_program():
    import concourse.bass as bass  # noqa: F401
    from concourse import bacc, mybir
    from concourse.tile import TileContext

    f32 = mybir.dt.float32
    AX = mybir.AxisListType.X
    OP = mybir.AluOpType
    AF = mybir.ActivationFunctionType

    nc = bacc.Bacc(trn_type="TRN2")

    xs = nc.declare_dram_parameter("xs", [BS, T], f32, isOutput=False)
    ys = nc.declare_dram_parameter("ys", [BS, T], f32, isOutput=False)
    whh = nc.declare_dram_parameter("whh", [H, H], f32, isOutput=False)
    wih = nc.declare_dram_parameter("wih", [H, 1], f32, isOutput=False)
    bih = nc.declare_dram_parameter("bih", [H], f32, isOutput=False)
    bhh = nc.declare_dram_parameter("bhh", [H], f32, isOutput=False)
    prior = nc.declare_dram_parameter("prior", [1], f32, isOutput=False)
    cs = nc.declare_dram_parameter("cs", [BS, T], f32, isOutput=True)
    ls = nc.declare_dram_parameter("ls", [BS, T], f32, isOutput=True)
    lp = nc.declare_dram_parameter("lp", [P], f32, isOutput=True)

    T1 = T + 1
    GC = G // NCHAIN  # groups per chain

    with TileContext(nc) as tc, ExitStack() as ctx:
        mp = ctx.enter_context(tc.tile_pool(name="main", bufs=1))
        sp = ctx.enter_context(tc.tile_pool(name="scratch", bufs=2))
        pp = ctx.enter_context(tc.tile_pool(name="psum", bufs=1, space="PSUM"))

        IN = mp.tile([P, 6 * G * T1], f32)
        Y = mp.tile([P, G * T], f32)
        LAM = mp.tile([P, G * T1], f32)  # latent, index shifted by one step
        TTL = mp.tile([P, G * T], f32)  # BCE select values, Ln'd in place
        CB = mp.tile([P, G * T], f32)  # corrects staging
        W6 = mp.tile([P, G * 6], f32)  # broadcast folded weights [i, j]
        ONE = mp.tile([P, G * C], f32)  # all-ones for Pool-side 1-x ops

        INp = IN.rearrange("p (j g t) -> p j g t", j=6, g=G)
        Yr = Y.rearrange("p (g t) -> p g t", g=G)
        LAMr = LAM.rearrange("p (g t) -> p g t", g=G)
        TTLr = TTL.rearrange("p (g t) -> p g t", g=G)
        CBr = CB.rearrange("p (g t) -> p g t", g=G)
        W6r = W6.rearrange("p (i j) -> p i j", i=H)

        # ---- init: fold weights and broadcast to all partitions ----
        whh_t = mp.tile([1, H * H], f32)
        wih_t = mp.tile([1, H], f32)
        bih_t = mp.tile([1, H], f32)
        bhh_t = mp.tile([1, H], f32)
        prior_t = mp.tile([1, 1], f32)
        nc.sync.dma_start(out=whh_t[:, :], in_=whh[:, :].flatten().unsqueeze(0))
        nc.sync.dma_start(out=wih_t[:, :], in_=wih[:, :].flatten().unsqueeze(0))
        nc.sync.dma_start(out=bih_t[:, :], in_=bih[:].unsqueeze(0))
        nc.sync.dma_start(out=bhh_t[:, :], in_=bhh[:].unsqueeze(0))
        nc.sync.dma_start(out=prior_t[:, :], in_=prior[:].unsqueeze(0))

        r24 = mp.tile([1, H * 6], f32)  # one-partition row of W6
        r24v = r24.rearrange("p (i j) -> p i j", i=H)
        whh_v = whh_t.rearrange("p (i j) -> p i j", i=H)
        nc.vector.tensor_scalar(
            out=r24v[:, :, 0:4], in0=whh_v, scalar1=2.0, scalar2=None, op0=OP.mult
        )
        nc.vector.tensor_scalar(
            out=r24v[:, :, 4], in0=wih_t[:, :], scalar1=0.0, scalar2=None, op0=OP.add
        )
        rowsum = mp.tile([1, H], f32)
        nc.vector.tensor_reduce(out=rowsum[:, :], in_=whh_v, axis=AX, op=OP.add)
        bsum = mp.tile([1, H], f32)
        nc.vector.tensor_tensor(
            out=bsum[:, :], in0=bih_t[:, :], in1=bhh_t[:, :], op=OP.add
        )
        nc.vector.tensor_tensor(
            out=r24v[:, :, 5], in0=bsum[:, :], in1=rowsum[:, :], op=OP.subtract
        )

        ones1 = mp.tile([1, P], f32)
        nc.vector.memset(ones1[:, :], 1.0)
        pw6 = pp.tile([P, H * 6], f32)
        nc.tensor.matmul(
            out=pw6[:, :], lhsT=ones1[:, :], rhs=r24[:, :], start=True, stop=True
        )
        nc.scalar.copy(out=W6[:, :], in_=pw6[:, :])

        # PE Matmult carries at most one sync wait; route prior through DVE so
        # both matmuls depend only on the DVE semaphore.
        prior_s = mp.tile([1, 1], f32)
        nc.vector.tensor_scalar(
            out=prior_s[:, :], in0=prior_t[:, :], scalar1=0.0, scalar2=None,
            op0=OP.add,
        )
        ppr = pp.tile([P, 1], f32)
        nc.tensor.matmul(
            out=ppr[:, :], lhsT=ones1[:, :], rhs=prior_s[:, :], start=True,
            stop=True,
        )
        nc.scalar.activation(
            out=LAMr[:, :, 0], in_=ppr[:, 0:1].broadcast_to((P, G)), func=AF.Sigmoid
        )

        nc.vector.memset(ONE[:, :], 1.0)

        # h'_0 = (tanh(0)+1)/2 = 0.5; const-1 plane for the bias term.
        nc.vector.memset(INp[:, 0:4, :, 0], 0.5)
        nc.vector.memset(INp[:, 5, :, 0:T], 1.0)

        # ---- bulk input DMAs (x into the j=4 plane, y into Y) ----
        for g in range(G):
            psl = slice(g * P, (g + 1) * P)
            nc.sync.dma_start(out=INp[:, 4, g, 0:T], in_=xs[psl, :])
            nc.sync.dma_start(out=Yr[:, g, :], in_=ys[psl, :])

        # ---- phase B (deferred per chunk): linear BKT scan + outputs ----
        def pviews(c, j):
            # p_t components for t in [t0, t0+C): state plane j at tau=t+1
            t0 = c * C
            return INp[:, j, :, t0 + 1 : t0 + C + 1]

        def phase_b_ops(c):
            t0 = c * C
            y_gt = Yr[:, :, t0 : t0 + C]
            lam_sh = LAMr[:, :, t0 : t0 + C]
            cb_ch = CBr[:, :, t0 : t0 + C]
            onev = ONE.rearrange("p (g t) -> p g t", g=G)

            def alloc(nm):
                t_ = sp.tile([P, G * C], f32, tag=f"bk_{nm}", name=f"bk_{nm}{c}")
                return t_.rearrange("p (g t) -> p g t", g=G)

            st = {}

            def op_a1():
                st["A"] = alloc("A")
                nc.gpsimd.tensor_tensor(
                    out=st["A"], in0=pviews(c, 0), in1=pviews(c, 1), op=OP.add
                )

            def op_a2():
                nc.gpsimd.tensor_tensor(
                    out=st["A"], in0=onev, in1=st["A"], op=OP.subtract
                )

            def op_b1():
                st["BC"] = alloc("BC")
                nc.gpsimd.tensor_tensor(
                    out=st["BC"], in0=pviews(c, 2), in1=pviews(c, 3), op=OP.add
                )

            def op_b2():
                nc.gpsimd.tensor_tensor(
                    out=st["BC"], in0=onev, in1=st["BC"], op=OP.subtract
                )

            def mk_scan(g):
                def op_scan():
                    nc.vector.tensor_tensor_scan(
                        out=LAMr[:, g, t0 + 1 : t0 + C + 1],
                        data0=st["A"][:, g, :],
                        data1=INp[:, 0, g, t0 + 1 : t0 + C + 1],
                        initial=LAMr[:, g, t0 : t0 + 1],
                        op0=OP.mult,
                        op1=OP.add,
                    )

                return op_scan

            def op_cm():
                nc.gpsimd.tensor_tensor(
                    out=cb_ch, in0=lam_sh, in1=st["BC"], op=OP.mult
                )

            def op_cc():
                nc.gpsimd.tensor_tensor(
                    out=cb_ch, in0=cb_ch, in1=pviews(c, 2), op=OP.add
                )

            def op_ym1():
                st["YM"] = alloc("YM")
                nc.gpsimd.tensor_tensor(
                    out=st["YM"], in0=y_gt, in1=onev, op=OP.subtract
                )

            def op_z1():
                st["Z1"] = alloc("Z1")
                nc.gpsimd.tensor_tensor(out=st["Z1"], in0=cb_ch, in1=y_gt,
                                        op=OP.mult)

            def op_cp():
                nc.gpsimd.tensor_tensor(
                    out=st["YM"], in0=cb_ch, in1=st["YM"], op=OP.add
                )

            def op_t2():
                nc.gpsimd.tensor_tensor(
                    out=st["Z1"], in0=st["Z1"], in1=st["Z1"], op=OP.add
                )

            def op_ttl():
                nc.gpsimd.tensor_tensor(
                    out=TTLr[:, :, t0 : t0 + C], in0=st["Z1"], in1=st["YM"],
                    op=OP.subtract,
                )

            ops = [op_a1, op_a2, op_b1, op_b2]
            ops += [mk_scan(g) for g in range(G)]
            ops += [op_cm, op_cc, op_ym1, op_z1, op_cp, op_t2, op_ttl]
            return ops

        def out_dmas(sec):
            t0 = sec * SEC
            tsl = slice(t0, t0 + SEC)
            dmas = []
            for g in range(G):
                psl = slice(g * P, (g + 1) * P)

                def mk(g=g, psl=psl):
                    def d():
                        nc.sync.dma_start(out=cs[psl, tsl], in_=CBr[:, g, tsl])
                        nc.sync.dma_start(
                            out=ls[psl, tsl],
                            in_=LAMr[:, g, t0 + 1 : t0 + SEC + 1],
                        )

                    return d

                dmas.append(mk())
            return dmas

        # ---- phase A: two interleaved RNN chains, phase B in the gaps ----
        w6c = [
            W6r[:, :, :].unsqueeze(1).broadcast_to((P, GC, H, 6))
            for _ in range(NCHAIN)
        ]
        pending = []
        for t in range(T):
            tmps, pres = [], []
            for ch in range(NCHAIN):
                g0 = ch * GC
                tmp = sp.tile([P, GC * H * 6], f32, tag=f"tmp{ch}",
                              name=f"tmp{ch}_{t}")
                tmpv = tmp.rearrange("p (g i j) -> p g i j", g=GC, i=H)
                in_exp = (
                    INp[:, :, g0 : g0 + GC, t]
                    .transpose([0, 2, 1])
                    .unsqueeze(2)
                    .broadcast_to((P, GC, H, 6))
                )
                nc.vector.tensor_tensor(out=tmpv, in0=in_exp, in1=w6c[ch],
                                        op=OP.mult)
                tmps.append(tmpv)
            for ch in range(NCHAIN):
                pre = sp.tile([P, GC * H], f32, tag=f"pre{ch}",
                              name=f"pre{ch}_{t}")
                prev = pre.rearrange("p (g i) -> p g i", g=GC)
                nc.vector.tensor_reduce(out=prev, in_=tmps[ch], axis=AX, op=OP.add)
                pres.append(prev)
            for ch in range(NCHAIN):
                g0 = ch * GC
                sig_out = INp[:, 0:4, g0 : g0 + GC, t + 1].transpose([0, 2, 1])
                nc.scalar.activation(
                    out=sig_out, in_=pres[ch], func=AF.Sigmoid, scale=2.0
                )
            # spread deferred work into the inter-step gaps
            if pending and (t % 4) == 2:
                pending.pop(0)()
            if (t + 1) % C == 0:
                while pending:
                    pending.pop(0)()
                pending = phase_b_ops((t + 1) // C - 1)
                if (t + 1) % SEC == 0 and (t + 1) > SEC:
                    pending += out_dmas((t + 1) // SEC - 2)
        while pending:
            pending.pop(0)()
        for d in out_dmas(T // SEC - 1):
            d()

        # ---- loss tail: sum(log(ttl)) per partition ----
        lsum = mp.tile([P, 1], f32)
        nc.scalar.activation(
            out=TTL[:, :], in_=TTL[:, :], func=AF.Ln, accum_out=lsum[:, :]
        )
        nc.sync.dma_start(out=lp[:].unsqueeze(0).transpose([1, 0]), in_=lsum[:, :])

    nc.finalize()
    return nc


def _get_program():
    global _PROGRAM
    if _PROGRAM is None:
        _PROGRAM = _build_program()
    return _PROGRAM


def kernel(x, y, prior, W_ih, W_hh, b_ih, b_hh):
    from concourse.bass_utils import run_bass_kernel_spmd

    nc = _get_program()
    x2 = np.ascontiguousarray(x.reshape(T, B))
    y2 = np.ascontiguousarray(y.reshape(T, B))
    in_maps = []
    for c in range(NCORES):
        sl = slice(c * BS, (c + 1) * BS)
        in_maps.append(
            {
                "xs": np.ascontiguousarray(x2[:, sl].T),
                "ys": np.ascontiguousarray(y2[:, sl].T),
                "whh": np.ascontiguousarray(W_hh),
                "wih": np.ascontiguousarray(W_ih),
                "bih": np.ascontiguousarray(b_ih),
                "bhh": np.ascontiguousarray(b_hh),
                "prior": np.ascontiguousarray(prior),
            }
        )
    res = run_bass_kernel_spmd(nc, in_maps, list(range(NCORES))).results
    corrects = np.empty((T, B, 1), np.float32)
    latents = np.empty((T, B, 1), np.float32)
    total = 0.0
    for c in range(NCORES):
        sl = slice(c * BS, (c + 1) * BS)
        corrects[:, sl, 0] = res[c]["cs"].T
        latents[:, sl, 0] = res[c]["ls"].T
        total += float(np.sum(res[c]["lp"], dtype=np.float64))
    loss = np.float32(-total / (T * B))
    return corrects, latents, loss
